# revision 40
# baseline (speedup 1.0000x reference)
"""Bilinear interpolation (affine grid sampling) Trainium2 Bass kernel, v4.

image [32,256,256,32] f32 + theta [32,6] f32 -> out [32,256,256,32] f32.
Data-parallel over batch: 4 samples per core on 8 cores.

v4 (active, VERSION=4) vs v3 (kept for A/B):
  - 256B gather elements: qimg4[u = y*256 + x] = img[y:y+2, x:x+2, :] fp16
    holds exactly the pixel's 4 corners (half of v3's 512B element). The
    65536-unit index exceeds int16, but the Q7 descriptor generator
    computes addr = base + SIGNED(idx)*stride (IVP_MULUSAN_2X32
    sign-extends), so in_ap = qimg4[32768:] with idx = y0*256+x0-32768
    spans the whole table. The firmware drops TRAILING negative indices
    (its padding convention), so a stream permutation (_perm4, involution,
    inverted host-side during unshard) pins a row-255 output pixel
    (y0=255 -> idx>=0; asserted host-side from the actual theta) at the
    last slot of every sub-gather.
  - 4-slot blend (7 DVE ops/call vs 11): no parity/column-split needed
    since the element has exactly the 2 needed columns.
  - overlap: idx replication DMAs issued first per sample on the ACT HWDGE
    queue with 4 reps buffers (never head-of-line blocks), gpool bufs=3,
    stores merged in call pairs on the sync queue.
  - gathers are the wall: SWDGE generation (~1.35us/instr, serialized on
    the Pool engine; sub-gather size is hard-capped at 1024 idx — bigger
    wedges the device on ring/idx-read limits) + HBM-latency-bound drain
    (~4-way concurrent via the 4 SWDGE queues).

v3 notes (still apply): fp16 blend with packed 2-byte stride-1 APs for DVE
2x_1p; weight-over-channel broadcast as [KB][16 x stride-0][2 x stride-1]
pair-duplicated planes; clip-first coordinate pipeline with int-cast floor
plus is_gt correction; converts/pair-dup on the Activation engine.
"""

import sys

sys.path.insert(0, "/opt/trn_rl_repo")

from contextlib import ExitStack

import numpy as np

import bass_rust
import concourse.bacc as bacc
import concourse.tile as tile
from concourse import mybir
from concourse.bass_utils import run_bass_kernel_spmd
from concourse.library_config import mlp

B_TOTAL = 32
N_CORES = 8
S = B_TOTAL // N_CORES      # 4 samples per core
H = W = 256
C = 32
HW = H * W                  # 65536
ELEM = 256                  # fp16 elements per gather element (512B)
NU = H * (W // 2)           # 32768 units per sample
P = 128
KB = 64                     # pixels per partition per call
NIDX = P * KB               # 8192 indices per call
NCALL = HW // NIDX          # 8 calls per sample
SUB = 1024                  # indices per dma_gather (SWDGE ring limit)
NSUB = NIDX // SUB          # 4 sub-gathers per call
Q = HW // P                 # 512 out-layout columns per sample
FW = HW // 16 // 2          # 2048 wrapped-f columns (two partition halves)

USE_MOD = False             # floor via AluOp.mod (x>=0); False -> int cast
                            # (mod is rejected by the walrus ISA checker)
OV = False                  # overlapping 256B-stride table: verified
                            # correct but ~4x slower gathers on HW (256B
                            # element alignment breaks the DMA fast path)

_COMPILED = {}

VERSION = 4                 # active kernel(): 3 = v3 (512B gather), 4 = v4

# ---- v4: 256B gather elements via signed-int16 index trick ----
# Table qimg4[u = y*256 + x] = img[y:y+2, x:x+2, :] fp16 (256B elements,
# 65536 units). int16 can only span 32768 units, but the Q7 descriptor
# generator computes addr = base + SIGNED(idx)*stride (IVP_MULUSAN_2X32
# sign-extends the int16), so passing in_ap = qimg4[32768:] and
# idx = u - 32768 addresses the full table. The firmware drops TRAILING
# negative indices (padding convention), so a stream permutation pins a
# row-255 output pixel (y0 = 255 with ~6-sigma certainty for near-identity
# theta; asserted host-side from the actual theta) at the end of every
# 1024-index sub-gather. The permutation is baked into the host grid
# tables and inverted on the host during unshard.
NU4 = H * W                 # 65536 table units per sample
ELEM4 = 128                 # fp16 elements per gather element (256B)
NSUB4 = NIDX // SUB         # 8 sub-gathers per call
TRUNC_CAST = False          # HW-tested: the ACT f32->i32 cast ROUNDS to
                            # nearest (rel err 7.7 without the fix), so
                            # floor(v>=0) keeps the is_gt round-up fix.


def _build_nc(mode="full"):
    """mode: "full" | "noblend" (gathers+stores only) | "nogather"
    (DVE pipeline on a static tile) | "io" (I/O surface only) |
    "blend1x" (blend with broadcast weights, no 2x packing) |
    "v4*" (routed to _build_nc_v4)."""
    if mode.startswith("v4"):
        return _build_nc_v4(mode)
    f32 = mybir.dt.float32
    f16 = mybir.dt.float16
    i32 = mybir.dt.int32
    i16 = mybir.dt.int16
    AF = mybir.AluOpType
    big_ring = mode == "nb_sub2048"
    nq = 2 if big_ring else 4
    nc = bacc.Bacc("TRN2", target_bir_lowering=False,
                   dynamic_dma_scratch_size=32768 if big_ring else 16384,
                   num_swdge_queues=nq)

    qshape = [NU + 1, ELEM // 2] if OV else [NU, ELEM]
    qimg = [nc.dram_tensor(f"qimg{b}", qshape, f16, kind="ExternalInput")
            for b in range(S)]
    xg_o_d = nc.dram_tensor("xg_o", [P, Q], f32, kind="ExternalInput")
    yg_o_d = nc.dram_tensor("yg_o", [P, Q], f32, kind="ExternalInput")
    xg_w_d = nc.dram_tensor("xg_w", [P, FW], f32, kind="ExternalInput")
    yg_w_d = nc.dram_tensor("yg_w", [P, FW], f32, kind="ExternalInput")
    th_o_d = nc.dram_tensor("th_o", [P, 6 * S], f32, kind="ExternalInput")
    th_w_d = nc.dram_tensor("th_w", [P, 6], f32, kind="ExternalInput")
    tok_d = nc.dram_tensor("tok", [P, 32], f32, kind="ExternalInput")
    out_d = nc.dram_tensor("out", [S, NCALL, P, KB, C], f16,
                           kind="ExternalOutput")
    tok_o_d = nc.dram_tensor("tok_out", [P, 32], f32, kind="ExternalOutput")

    V = nc.vector
    A = nc.scalar
    Copy = mybir.ActivationFunctionType.Copy
    Ident = mybir.ActivationFunctionType.Identity

    with tile.TileContext(nc) as tc, ExitStack() as ctx:
        nc.gpsimd.load_library(mlp)

        singles = ctx.enter_context(tc.tile_pool(name="singles", bufs=1))
        idx16w = singles.tile([P, FW], i16)
        xg_o = singles.tile([P, Q], f32)
        yg_o = singles.tile([P, Q], f32)
        th_o = singles.tile([P, 6 * S], f32)
        th_w = singles.tile([P, 6], f32)
        tho2 = singles.tile([P, 6 * S], f32)
        thw2 = singles.tile([P, 6], f32)
        tokt = singles.tile([P, 32], f32)
        nc.sync.dma_start(out=xg_o[:], in_=xg_o_d[:])
        nc.sync.dma_start(out=yg_o[:], in_=yg_o_d[:])
        nc.sync.dma_start(out=th_o[:], in_=th_o_d[:])
        nc.sync.dma_start(out=th_w[:], in_=th_w_d[:])
        nc.sync.dma_start(out=tokt[:], in_=tok_d[:])
        nc.sync.dma_start(out=tok_o_d[:], in_=tokt[:])

        # theta prescale: coefficients *128; constant cols (2,5) also +128,
        # so x = xg*t0' + (yg*t1' + t2') is already in pixel units.
        V.tensor_scalar(out=tho2[:], in0=th_o[:], scalar1=128.0,
                        scalar2=None, op0=AF.mult)
        V.tensor_scalar(out=thw2[:], in0=th_w[:], scalar1=128.0,
                        scalar2=None, op0=AF.mult)
        for col in (2, 5):
            V.tensor_scalar(out=thw2[:, col:col + 1],
                            in0=thw2[:, col:col + 1], scalar1=128.0,
                            scalar2=None, op0=AF.add)
            for b in range(S):
                V.tensor_scalar(out=tho2[:, 6 * b + col:6 * b + col + 1],
                                in0=tho2[:, 6 * b + col:6 * b + col + 1],
                                scalar1=128.0, scalar2=None, op0=AF.add)

        def floor_nn(pool, v, tag, width):
            """floor(v) for v >= 0. Returns an f32 tile."""
            if USE_MOD:
                fr = pool.tile([P, width], f32, tag="scrF", name="scrF")
                V.tensor_scalar(out=fr[:], in0=v[:], scalar1=1.0,
                                scalar2=None, op0=AF.mod)
                o = pool.tile([P, width], f32, tag=tag, name=tag)
                V.tensor_tensor(out=o[:], in0=v[:], in1=fr[:], op=AF.subtract)
                return o
            vi = pool.tile([P, width], i32, tag="scrI", name="scrI")
            V.tensor_copy(out=vi[:], in_=v[:])
            o = pool.tile([P, width], f32, tag=tag, name=tag)
            V.tensor_copy(out=o[:], in_=vi[:])
            g = pool.tile([P, width], f32, tag="scrF", name="scrF")
            V.tensor_tensor(out=g[:], in0=o[:], in1=v[:], op=AF.is_gt)
            V.tensor_tensor(out=o[:], in0=o[:], in1=g[:], op=AF.subtract)
            return o

        if mode == "io":
            iot = singles.tile([P, ELEM], f16)
            for b in range(S):
                nc.sync.dma_start(out=iot[0:1, :], in_=qimg[b][0:1, :])
            xgw0 = singles.tile([P, FW], f32)
            nc.sync.dma_start(out=xgw0[:], in_=xg_w_d[:])
            nc.sync.dma_start(out=xgw0[:], in_=yg_w_d[:])
            ioo = singles.tile([P, KB, C], f16)
            V.memset(ioo[:], 0.0)
            nc.sync.dma_start(out=out_d[0, 0], in_=ioo[:])

        HF = FW // 2
        if mode != "io":
            # ---- wrapped int16 index pipeline, in column-halves so the
            # first gathers can start before the whole pipeline finishes
            with ExitStack() as wctx:
                wpool = wctx.enter_context(tc.tile_pool(name="wpool", bufs=2))
                tw = [thw2[:, k:k + 1] for k in range(6)]
                for wh in range(2):
                    wsl = slice(HF * wh, HF * (wh + 1))

                    def wt(tag, dt=f32):
                        return wpool.tile([P, HF], dt, tag=tag, name=tag)

                    xgw = wt("xgw")
                    ygw = wt("ygw")
                    nc.sync.dma_start(out=xgw[:], in_=xg_w_d[:, wsl])
                    nc.sync.dma_start(out=ygw[:], in_=yg_w_d[:, wsl])

                    def w_coord(t0, t1, t2, tag):
                        a = wt("wA")
                        b = wt("wB")
                        A.activation(out=a[:], in_=xgw[:], func=Copy,
                                     scale=t0)
                        A.activation(out=b[:], in_=ygw[:], func=Ident,
                                     scale=t1, bias=t2)
                        v = wt(tag)
                        V.tensor_tensor(out=v[:], in0=a[:], in1=b[:],
                                        op=AF.add)
                        V.tensor_scalar(out=v[:], in0=v[:], scalar1=0.0,
                                        scalar2=255.5, op0=AF.max,
                                        op1=AF.min)
                        return v

                    xwc = w_coord(tw[0], tw[1], tw[2], "wX")
                    ywc = w_coord(tw[3], tw[4], tw[5], "wY")
                    x0w = floor_nn(wpool, xwc, "wA", HF)
                    y0w = floor_nn(wpool, ywc, "wB", HF)
                    xh = wt("wX")
                    V.tensor_scalar(out=xh[:], in0=x0w[:], scalar1=0.5,
                                    scalar2=None, op0=AF.mult)
                    xh2 = floor_nn(wpool, xh, "wY", HF)
                    idxf = wt("wA2")
                    A.activation(out=idxf[:], in_=y0w[:], func=Copy,
                                 scale=128.0)
                    V.tensor_tensor(out=idxf[:], in0=idxf[:], in1=xh2[:],
                                    op=AF.add)
                    V.tensor_copy(out=idx16w[:, wsl], in_=idxf[:])

        # ---- steady-state pools ----
        cpool = ctx.enter_context(tc.tile_pool(name="cpool", bufs=1))
        hpool = ctx.enter_context(tc.tile_pool(name="hpool", bufs=1))
        mpool = ctx.enter_context(tc.tile_pool(name="mpool", bufs=2))
        reps = ctx.enter_context(tc.tile_pool(name="reps", bufs=2))
        gpool = ctx.enter_context(tc.tile_pool(name="gpool", bufs=2))
        opool = ctx.enter_context(tc.tile_pool(name="opool", bufs=3))

        def ct(tag):
            return cpool.tile([P, Q], f32, tag=tag, name=tag)

        def ht(tag):
            return hpool.tile([P, Q], f16, tag=tag, name=tag)

        for b in range(S if mode != "io" else 0):
            t = [tho2[:, 6 * b + k:6 * b + k + 1] for k in range(6)]

            if not mode.startswith("nb") and mode != "noblend":
                # f32 coordinate stage
                def coord(t0, t1, t2, tag):
                    a = ct("scrA")
                    bb = ct("scrB")
                    A.activation(out=a[:], in_=xg_o[:], func=Copy,
                                 scale=t0)
                    A.activation(out=bb[:], in_=yg_o[:], func=Ident,
                                 scale=t1, bias=t2)
                    v = ct(tag)
                    V.tensor_tensor(out=v[:], in0=a[:], in1=bb[:], op=AF.add)
                    return v

                x = coord(t[0], t[1], t[2], "x")
                y = coord(t[3], t[4], t[5], "y")
                xc = ct("xc")
                V.tensor_scalar(out=xc[:], in0=x[:], scalar1=0.0,
                                scalar2=255.5, op0=AF.max, op1=AF.min)
                yc = ct("yc")
                V.tensor_scalar(out=yc[:], in0=y[:], scalar1=0.0,
                                scalar2=255.5, op0=AF.max, op1=AF.min)
                x0c = floor_nn(cpool, xc, "x0c", Q)
                y0c = floor_nn(cpool, yc, "y0c", Q)
                # x1c = clip(trunc(x)+1): from clipped x0c this is min(x0c+1,255)
                # except x <= -1 where the reference collapses to 0; the is_le
                # mask subtracts the 1 back exactly in that case.
                x1c = ct("x1c")
                V.tensor_scalar(out=x1c[:], in0=x0c[:], scalar1=1.0,
                                scalar2=255.0, op0=AF.add, op1=AF.min)
                mneg = ct("scrA")
                V.tensor_scalar(out=mneg[:], in0=x[:], scalar1=-1.0,
                                scalar2=None, op0=AF.is_le)
                V.tensor_tensor(out=x1c[:], in0=x1c[:], in1=mneg[:],
                                op=AF.subtract)
                y1c = ct("y1c")
                V.tensor_scalar(out=y1c[:], in0=y0c[:], scalar1=1.0,
                                scalar2=255.0, op0=AF.add, op1=AF.min)
                V.tensor_scalar(out=mneg[:], in0=y[:], scalar1=-1.0,
                                scalar2=None, op0=AF.is_le)
                V.tensor_tensor(out=y1c[:], in0=y1c[:], in1=mneg[:],
                                op=AF.subtract)
                u1 = ct("u1")
                V.tensor_tensor(out=u1[:], in0=x1c[:], in1=x[:], op=AF.subtract)
                u0 = ct("u0")
                V.tensor_tensor(out=u0[:], in0=x[:], in1=x0c[:], op=AF.subtract)
                v1 = ct("v1")
                V.tensor_tensor(out=v1[:], in0=y1c[:], in1=y[:], op=AF.subtract)
                v0 = ct("v0")
                V.tensor_tensor(out=v0[:], in0=y[:], in1=y0c[:], op=AF.subtract)

                # converts to fp16 on the Activation engine
                u1h, u0h, v1h, v0h = ht("u1h"), ht("u0h"), ht("v1h"), ht("v0h")
                x0h, x1h, y0h, y1h = ht("x0h"), ht("x1h"), ht("y0h"), ht("y1h")
                for dst, src in ((u1h, u1), (u0h, u0), (v1h, v1), (v0h, v0),
                                 (x0h, x0c), (x1h, x1c), (y0h, y0c), (y1h, y1c)):
                    A.activation(out=dst[:], in_=src[:], func=Copy)

                # fp16 stage
                def tt(o, i0, i1, op):
                    V.tensor_tensor(out=o[:], in0=i0[:], in1=i1[:], op=op)

                cx, cy = ht("cx"), ht("cy")
                tt(cx, x1h, x0h, AF.is_equal)
                tt(cy, y1h, y0h, AF.is_equal)
                if USE_MOD:
                    par = ht("par")
                    V.tensor_scalar(out=par[:], in0=x0h[:], scalar1=2.0,
                                    scalar2=None, op0=AF.mod)
                else:
                    phf = ct("scrA")
                    V.tensor_scalar(out=phf[:], in0=x0c[:], scalar1=0.5,
                                    scalar2=None, op0=AF.mult)
                    ph2 = floor_nn(cpool, phf, "scrB", Q)
                    V.tensor_scalar(out=ph2[:], in0=ph2[:], scalar1=-2.0,
                                    scalar2=None, op0=AF.mult)
                    V.tensor_tensor(out=ph2[:], in0=ph2[:], in1=x0c[:],
                                    op=AF.add)
                    par = ht("par")
                    A.activation(out=par[:], in_=ph2[:], func=Copy)

                # fold weights of clipped-away neighbors directly on the u/v
                # factors: u1' = u1 + u0*cx, u0' = u0*(1-cx) (and same for v)
                # reproduces the reference's clipped-corner weight collapse.
                ft = ht("ft")
                tt(ft, u0h, cx, AF.mult)
                tt(u1h, u1h, ft, AF.add)
                tt(u0h, u0h, ft, AF.subtract)
                tt(ft, v0h, cy, AF.mult)
                tt(v1h, v1h, ft, AF.add)
                tt(v0h, v0h, ft, AF.subtract)
                # parity column split on the x-weights: block-local cols get
                # U0 = u1*(1-par), U1 = u1*par + u0*(1-par), U2 = u0*par
                pu, U0, U1, U2 = ht("pu"), ht("U0"), ht("U1"), ht("U2")
                tt(pu, u1h, par, AF.mult)
                tt(U0, u1h, pu, AF.subtract)
                tt(U2, u0h, par, AF.mult)
                tt(U1, u0h, U2, AF.subtract)
                tt(U1, U1, pu, AF.add)
                # six premultiplied planes: m[r*3+c] = U_c * v_r (v1 = top row)
                m0, m1, m2 = ht("m0"), ht("m1"), ht("m2")
                m3, m4, m5 = ht("m3"), ht("m4"), ht("m5")
                tt(m0, U0, v1h, AF.mult)
                tt(m1, U1, v1h, AF.mult)
                tt(m2, U2, v1h, AF.mult)
                tt(m3, U0, v0h, AF.mult)
                tt(m4, U1, v0h, AF.mult)
                tt(m5, U2, v0h, AF.mult)

                # pair-duplicated planes (ACT engine): mdup[p, q, e] = m[p, q]
                ms = [m0, m1, m2, m3, m4, m5]
                mdup = [mpool.tile([P, Q, 2], f16, tag=f"md{k}", name=f"md{k}")
                        for k in range(6)]
                for k in range(6):
                    for e in range(2):
                        A.activation(out=mdup[k][:, :, e:e + 1],
                                     in_=ms[k][:, :, None], func=Copy)

            if mode != "nogather":
                # replicate this sample's wrapped idx to all 8 16-partition
                # groups (both halves -> [P, 2*FW])
                idx_rep = reps.tile([P, 2 * FW], i16, tag="idx_rep",
                                    name="idx_rep")
                for g8 in range(8):
                    for wh in range(2):
                        wsl = slice(HF * wh, HF * (wh + 1))
                        nc.sync.dma_start(
                            out=idx_rep[16 * g8:16 * g8 + 16, wsl],
                            in_=idx16w[16 * b:16 * b + 16, wsl])
                        nc.sync.dma_start(
                            out=idx_rep[16 * g8:16 * g8 + 16,
                                        FW + HF * wh:FW + HF * (wh + 1)],
                            in_=idx16w[64 + 16 * b:64 + 16 * b + 16, wsl])

            if mode == "nogather" and b == 0:
                gts = singles.tile([P, KB, ELEM], f16)
                V.memset(gts[:], 0.0)

            for j in range(NCALL):
                if mode == "nogather":
                    gt_t = gts
                else:
                    sub = 2048 if mode == "nb_sub2048" else SUB
                    nsub = NIDX // sub
                    el = ELEM // 2 if mode == "nb_half" else ELEM
                    if OV:
                        qap = qimg[b][:].copy()
                        qap.ap = bass_rust.VecI64Pair(
                            [(ELEM // 2, NU), (1, ELEM)])
                    else:
                        qap = None
                    gt_t = gpool.tile([P, KB, el], f16, tag="gt",
                                      name="gt")
                    kbsub = sub // P        # KB rows per sub-gather
                    csub = sub // 16        # idx cols per sub-gather
                    for c8 in range(nsub):
                        nc.gpsimd.dma_gather(
                            out_ap=gt_t[:, kbsub * c8:kbsub * (c8 + 1), :],
                            in_ap=qap if OV else (
                                qimg[b][:] if el == ELEM
                                else qimg[b][:].rearrange(
                                    "n (a b) -> (n a) b", a=2)),
                            idxs_ap=idx_rep[:, 512 * j + csub * c8:
                                            512 * j + csub * (c8 + 1)],
                            num_idxs=sub,
                            num_idxs_reg=sub,
                            elem_size=el,
                            elem_step=ELEM // 2 if OV else None,
                            queue_num=((j * nsub + c8) % nq
                                       if nq > 1 else 0),
                        )

                if mode == "noblend" or mode.startswith("nb"):
                    nc.sync.dma_start(out=out_d[b, j],
                                      in_=gt_t[:, :, 0:32])
                    continue

                ot = opool.tile([P, KB, C], f16, tag="ot", name="ot")
                tm = opool.tile([P, KB, C], f16, tag="tm", name="tm")
                csl = slice(KB * j, KB * j + KB)

                def msl(k):
                    return mdup[k][:, csl, None, :].to_broadcast(
                        [P, KB, 16, 2])

                def msl1(k):
                    return mdup[k][:, csl, 0:1].to_broadcast([P, KB, C])

                def gsl(r, cc, split):
                    if OV:
                        off = 128 + r * 64 if cc == 2 else r * 64 + cc * 32
                    else:
                        off = (4 * r + cc) * 32
                    ap = gt_t[:, :, off:off + 32]
                    return ap.rearrange("p k (a b) -> p k a b", b=2) \
                        if split else ap

                def o4(ap):
                    return ap.rearrange("p k (a b) -> p k a b", b=2)

                split = mode != "blend1x"
                wsl = msl if split else msl1
                slots = [(0, 0, 0), (1, 0, 1), (2, 0, 2),
                         (3, 1, 0), (4, 1, 1), (5, 1, 2)]
                k0, r0, c0 = slots[0]
                V.tensor_tensor(out=o4(ot[:]) if split else ot[:],
                                in0=gsl(r0, c0, split),
                                in1=wsl(k0), op=AF.mult)
                for k, r, cc in slots[1:]:
                    V.tensor_tensor(out=o4(tm[:]) if split else tm[:],
                                    in0=gsl(r, cc, split),
                                    in1=wsl(k), op=AF.mult)
                    V.tensor_tensor(out=ot[:], in0=ot[:], in1=tm[:],
                                    op=AF.add)

                nc.sync.dma_start(out=out_d[b, j], in_=ot[:])

    nc.compile()
    return nc


def _build_nc_v4(mode="v4"):
    """v4: 256B gather elements (signed-idx trick), 4-slot blend.

    mode: "v4" (full) | "v4nb" (gathers+stores only) | "v4ng" (no gather:
    DVE pipeline + blend on a static tile). Optional suffix "_sNNNN" sets
    the sub-gather size (multiple of 1024; permutation keeps every
    1024-boundary safe), e.g. "v4_s4096", "v4nb_s2048"."""
    f32 = mybir.dt.float32
    f16 = mybir.dt.float16
    i32 = mybir.dt.int32
    i16 = mybir.dt.int16
    AF = mybir.AluOpType
    sub = SUB
    zero_idx = False      # memset idx (locality probe) instead of pipeline
    one_queue = False     # all gathers on queue 0
    no_store = False      # skip output stores
    multi_packet = False  # single_packet=False on gathers
    p5 = False            # [1792]*4 + [1024] sub pattern (5 gathers/call)
    blend1x = False       # unpacked blend APs (forces DVE 1x mode)
    while "_" in mode:
        mode, _, s = mode.rpartition("_")
        if s.startswith("s") and s[1:].isdigit():
            sub = int(s[1:])
        elif s == "z":
            zero_idx = True
        elif s == "q1":
            one_queue = True
        elif s == "g":
            no_store = True
        elif s == "sp0":
            multi_packet = True
        elif s == "p5":
            p5 = True
        elif s == "b1":
            blend1x = True
        else:
            raise ValueError(s)
    assert sub % 1024 == 0 and NIDX % sub == 0
    call_subs = [1792] * 4 + [1024] if p5 else [sub] * (NIDX // sub)
    # (offset, size) per sub-gather within a call; permutation pins a safe
    # pixel at each end (see _EXTRA_ENDS)
    sub_offs = []
    off = 0
    for sz in call_subs:
        sub_offs.append((off, sz))
        off += sz
    assert off == NIDX and all(sz % 128 == 0 for _, sz in sub_offs)
    nq = 4
    # Ring capacity seems to need ~2*64*(sub/16+1) bytes/partition (tx+rx).
    scratch = max(16384, 256 * sub // 16)
    nc = bacc.Bacc("TRN2", target_bir_lowering=False,
                   dynamic_dma_scratch_size=scratch, num_swdge_queues=nq)

    qimg = [nc.dram_tensor(f"qimg{b}", [NU4, ELEM4], f16, kind="ExternalInput")
            for b in range(S)]
    xg_o_d = nc.dram_tensor("xg_o", [P, Q], f32, kind="ExternalInput")
    yg_o_d = nc.dram_tensor("yg_o", [P, Q], f32, kind="ExternalInput")
    xg_w_d = nc.dram_tensor("xg_w", [P, FW], f32, kind="ExternalInput")
    yg_w_d = nc.dram_tensor("yg_w", [P, FW], f32, kind="ExternalInput")
    th_o_d = nc.dram_tensor("th_o", [P, 6 * S], f32, kind="ExternalInput")
    th_w_d = nc.dram_tensor("th_w", [P, 6], f32, kind="ExternalInput")
    tok_d = nc.dram_tensor("tok", [P, 32], f32, kind="ExternalInput")
    out_d = nc.dram_tensor("out", [S, NCALL, P, KB, C], f16,
                           kind="ExternalOutput")
    tok_o_d = nc.dram_tensor("tok_out", [P, 32], f32, kind="ExternalOutput")

    V = nc.vector
    A = nc.scalar
    Copy = mybir.ActivationFunctionType.Copy
    Ident = mybir.ActivationFunctionType.Identity

    with tile.TileContext(nc) as tc, ExitStack() as ctx:
        nc.gpsimd.load_library(mlp)

        singles = ctx.enter_context(tc.tile_pool(name="singles", bufs=1))
        idx16w = singles.tile([P, FW], i16)
        xg_o = singles.tile([P, Q], f32)
        yg_o = singles.tile([P, Q], f32)
        th_o = singles.tile([P, 6 * S], f32)
        th_w = singles.tile([P, 6], f32)
        tho2 = singles.tile([P, 6 * S], f32)
        thw2 = singles.tile([P, 6], f32)
        tokt = singles.tile([P, 32], f32)
        bm32k = singles.tile([P, 1], f32)
        nc.sync.dma_start(out=xg_o[:], in_=xg_o_d[:])
        nc.sync.dma_start(out=yg_o[:], in_=yg_o_d[:])
        nc.sync.dma_start(out=th_o[:], in_=th_o_d[:])
        nc.sync.dma_start(out=th_w[:], in_=th_w_d[:])
        nc.sync.dma_start(out=tokt[:], in_=tok_d[:])
        nc.sync.dma_start(out=tok_o_d[:], in_=tokt[:])
        V.memset(bm32k[:], -32768.0)

        # theta prescale: coefficients *128; constant cols (2,5) also +128,
        # so x = xg*t0' + (yg*t1' + t2') is already in pixel units.
        V.tensor_scalar(out=tho2[:], in0=th_o[:], scalar1=128.0,
                        scalar2=None, op0=AF.mult)
        V.tensor_scalar(out=thw2[:], in0=th_w[:], scalar1=128.0,
                        scalar2=None, op0=AF.mult)
        for col in (2, 5):
            V.tensor_scalar(out=thw2[:, col:col + 1],
                            in0=thw2[:, col:col + 1], scalar1=128.0,
                            scalar2=None, op0=AF.add)
            for b in range(S):
                V.tensor_scalar(out=tho2[:, 6 * b + col:6 * b + col + 1],
                                in0=tho2[:, 6 * b + col:6 * b + col + 1],
                                scalar1=128.0, scalar2=None, op0=AF.add)

        def floor_nn(pool, v, tag, width):
            """floor(v) for v >= 0. Returns an f32 tile.

            The two dtype-cast copies run on ACT (own SBUF port): DVE ops
            block SWDGE descriptor generation via the shared port pair, so
            every DVE cycle here adds ~1:1 to the gather-bound total."""
            vi = pool.tile([P, width], i32, tag="scrI", name="scrI")
            A.activation(out=vi[:], in_=v[:], func=Copy)
            o = pool.tile([P, width], f32, tag=tag, name=tag)
            A.activation(out=o[:], in_=vi[:], func=Copy)
            if not TRUNC_CAST:
                g = pool.tile([P, width], f32, tag="scrF", name="scrF")
                V.tensor_tensor(out=g[:], in0=o[:], in1=v[:], op=AF.is_gt)
                V.tensor_tensor(out=o[:], in0=o[:], in1=g[:], op=AF.subtract)
            return o

        HF = FW // 2
        # ---- wrapped int16 index pipeline, in column-halves ----
        with ExitStack() as wctx:
            wpool = wctx.enter_context(tc.tile_pool(name="wpool", bufs=2))
            tw = [thw2[:, k:k + 1] for k in range(6)]
            for wh in range(2):
                wsl = slice(HF * wh, HF * (wh + 1))

                def wt(tag, dt=f32):
                    return wpool.tile([P, HF], dt, tag=tag, name=tag)

                xgw = wt("xgw")
                ygw = wt("ygw")
                nc.sync.dma_start(out=xgw[:], in_=xg_w_d[:, wsl])
                nc.sync.dma_start(out=ygw[:], in_=yg_w_d[:, wsl])

                def w_coord(t0, t1, t2, tag):
                    a = wt("wA")
                    b = wt("wB")
                    A.activation(out=a[:], in_=xgw[:], func=Copy,
                                 scale=t0)
                    A.activation(out=b[:], in_=ygw[:], func=Ident,
                                 scale=t1, bias=t2)
                    v = wt(tag)
                    V.tensor_tensor(out=v[:], in0=a[:], in1=b[:],
                                    op=AF.add)
                    V.tensor_scalar(out=v[:], in0=v[:], scalar1=0.0,
                                    scalar2=255.5, op0=AF.max,
                                    op1=AF.min)
                    return v

                xwc = w_coord(tw[0], tw[1], tw[2], "wX")
                ywc = w_coord(tw[3], tw[4], tw[5], "wY")
                x0w = floor_nn(wpool, xwc, "wA", HF)
                y0w = floor_nn(wpool, ywc, "wB", HF)
                # idx = y0*256 + x0 - 32768 (always in [-32768, 32767])
                idxf = wt("wA2")
                A.activation(out=idxf[:], in_=y0w[:], func=Ident,
                             scale=256.0, bias=bm32k[:])
                V.tensor_tensor(out=idxf[:], in0=idxf[:], in1=x0w[:],
                                op=AF.add)
                A.activation(out=idx16w[:, wsl], in_=idxf[:], func=Copy)

        # ---- steady-state pools ----
        cpool = ctx.enter_context(tc.tile_pool(name="cpool", bufs=1))
        hpool = ctx.enter_context(tc.tile_pool(name="hpool", bufs=1))
        mpool = ctx.enter_context(tc.tile_pool(name="mpool", bufs=2))
        reps = ctx.enter_context(tc.tile_pool(name="reps", bufs=4))
        gpool = ctx.enter_context(tc.tile_pool(name="gpool", bufs=4))
        opool = ctx.enter_context(tc.tile_pool(name="opool", bufs=3))

        def ct(tag):
            return cpool.tile([P, Q], f32, tag=tag, name=tag)

        def ht(tag):
            return hpool.tile([P, Q], f16, tag=tag, name=tag)

        for b in range(S):
            t = [tho2[:, 6 * b + k:6 * b + k + 1] for k in range(6)]

            if mode != "v4ng":
                # replicate this sample's wrapped idx to all 8 16-partition
                # groups (both halves -> [P, 2*FW]). Issued FIRST in each
                # sample's program order, on ACT's HWDGE queue, with enough
                # reps buffers that no tile-recycle wait can head-of-line
                # block the ACT queue.
                idx_rep = reps.tile([P, 2 * FW], i16, tag="idx_rep",
                                    name="idx_rep")
                if zero_idx:
                    V.memset(idx_rep[:], 0)
                else:
                    for g8 in range(8):
                        for wh in range(2):
                            wsl = slice(HF * wh, HF * (wh + 1))
                            nc.scalar.dma_start(
                                out=idx_rep[16 * g8:16 * g8 + 16, wsl],
                                in_=idx16w[16 * b:16 * b + 16, wsl])
                            nc.scalar.dma_start(
                                out=idx_rep[16 * g8:16 * g8 + 16,
                                            FW + HF * wh:FW + HF * (wh + 1)],
                                in_=idx16w[64 + 16 * b:64 + 16 * b + 16, wsl])

            if mode != "v4nb":
                # f32 coordinate stage
                def coord(t0, t1, t2, tag):
                    a = ct("scrA")
                    bb = ct("scrB")
                    A.activation(out=a[:], in_=xg_o[:], func=Copy,
                                 scale=t0)
                    A.activation(out=bb[:], in_=yg_o[:], func=Ident,
                                 scale=t1, bias=t2)
                    v = ct(tag)
                    V.tensor_tensor(out=v[:], in0=a[:], in1=bb[:], op=AF.add)
                    return v

                x = coord(t[0], t[1], t[2], "x")
                y = coord(t[3], t[4], t[5], "y")
                xc = ct("xc")
                V.tensor_scalar(out=xc[:], in0=x[:], scalar1=0.0,
                                scalar2=255.5, op0=AF.max, op1=AF.min)
                yc = ct("yc")
                V.tensor_scalar(out=yc[:], in0=y[:], scalar1=0.0,
                                scalar2=255.5, op0=AF.max, op1=AF.min)
                x0c = floor_nn(cpool, xc, "x0c", Q)
                y0c = floor_nn(cpool, yc, "y0c", Q)
                # x1c = clip(trunc(x)+1): min(x0c+1,255) except x <= -1 where
                # the reference collapses to 0; is_le mask fixes that case.
                x1c = ct("x1c")
                V.tensor_scalar(out=x1c[:], in0=x0c[:], scalar1=1.0,
                                scalar2=255.0, op0=AF.add, op1=AF.min)
                mneg = ct("scrA")
                V.tensor_scalar(out=mneg[:], in0=x[:], scalar1=-1.0,
                                scalar2=None, op0=AF.is_le)
                V.tensor_tensor(out=x1c[:], in0=x1c[:], in1=mneg[:],
                                op=AF.subtract)
                y1c = ct("y1c")
                V.tensor_scalar(out=y1c[:], in0=y0c[:], scalar1=1.0,
                                scalar2=255.0, op0=AF.add, op1=AF.min)
                V.tensor_scalar(out=mneg[:], in0=y[:], scalar1=-1.0,
                                scalar2=None, op0=AF.is_le)
                V.tensor_tensor(out=y1c[:], in0=y1c[:], in1=mneg[:],
                                op=AF.subtract)
                u1 = ct("u1")
                V.tensor_tensor(out=u1[:], in0=x1c[:], in1=x[:], op=AF.subtract)
                u0 = ct("u0")
                V.tensor_tensor(out=u0[:], in0=x[:], in1=x0c[:], op=AF.subtract)
                v1 = ct("v1")
                V.tensor_tensor(out=v1[:], in0=y1c[:], in1=y[:], op=AF.subtract)
                v0 = ct("v0")
                V.tensor_tensor(out=v0[:], in0=y[:], in1=y0c[:], op=AF.subtract)

                # converts to fp16 on the Activation engine
                u1h, u0h, v1h, v0h = ht("u1h"), ht("u0h"), ht("v1h"), ht("v0h")
                x0h, x1h, y0h, y1h = ht("x0h"), ht("x1h"), ht("y0h"), ht("y1h")
                for dst, src in ((u1h, u1), (u0h, u0), (v1h, v1), (v0h, v0),
                                 (x0h, x0c), (x1h, x1c), (y0h, y0c), (y1h, y1c)):
                    A.activation(out=dst[:], in_=src[:], func=Copy)

                def tt(o, i0, i1, op):
                    V.tensor_tensor(out=o[:], in0=i0[:], in1=i1[:], op=op)

                cx, cy = ht("cx"), ht("cy")
                tt(cx, x1h, x0h, AF.is_equal)
                tt(cy, y1h, y0h, AF.is_equal)
                # fold weights of clipped-away neighbors onto the kept ones
                ft = ht("ft")
                tt(ft, u0h, cx, AF.mult)
                tt(u1h, u1h, ft, AF.add)
                tt(u0h, u0h, ft, AF.subtract)
                tt(ft, v0h, cy, AF.mult)
                tt(v1h, v1h, ft, AF.add)
                tt(v0h, v0h, ft, AF.subtract)
                # four premultiplied planes: m[r*2+c], r=row (v1=top), c=col
                m0, m1, m2, m3 = ht("m0"), ht("m1"), ht("m2"), ht("m3")
                tt(m0, u1h, v1h, AF.mult)
                tt(m1, u0h, v1h, AF.mult)
                tt(m2, u1h, v0h, AF.mult)
                tt(m3, u0h, v0h, AF.mult)

                # pair-duplicated planes (ACT engine): mdup[p, q, e] = m[p, q]
                ms = [m0, m1, m2, m3]
                mdup = [mpool.tile([P, Q, 2], f16, tag=f"md{k}", name=f"md{k}")
                        for k in range(4)]
                for k in range(4):
                    for e in range(2):
                        A.activation(out=mdup[k][:, :, e:e + 1],
                                     in_=ms[k][:, :, None], func=Copy)

            if mode == "v4ng" and b == 0:
                gts = singles.tile([P, KB, ELEM4], f16)
                V.memset(gts[:], 0.0)

            for j in range(NCALL):
                if mode == "v4ng":
                    gt_t = gts
                else:
                    gt_t = gpool.tile([P, KB, ELEM4], f16, tag="gt",
                                      name="gt")
                    for c8, (soff, ssz) in enumerate(sub_offs):
                        nc.gpsimd.dma_gather(
                            out_ap=gt_t[:, soff // P:(soff + ssz) // P, :],
                            in_ap=qimg[b][NU4 // 2:, :],
                            idxs_ap=idx_rep[:, 512 * j + soff // 16:
                                            512 * j + (soff + ssz) // 16],
                            num_idxs=ssz,
                            num_idxs_reg=ssz,
                            elem_size=ELEM4,
                            single_packet=not multi_packet,
                            queue_num=0 if one_queue
                            else (j * len(sub_offs) + c8) % nq,
                        )

                if mode == "v4nb":
                    if not no_store:
                        nc.sync.dma_start(out=out_d[b, j],
                                          in_=gt_t[:, :, 0:32])
                    continue

                # paired output tile: two calls share one store DMA
                if j % 2 == 0:
                    ot2 = opool.tile([P, 2, KB, C], f16, tag="ot2",
                                     name="ot2")
                ot = ot2[:, j % 2]
                tm = opool.tile([P, KB, C], f16, tag="tm", name="tm")
                csl = slice(KB * j, KB * j + KB)

                def msl(k):
                    if blend1x:
                        return mdup[k][:, csl, 0:1].to_broadcast([P, KB, C])
                    return mdup[k][:, csl, None, :].to_broadcast(
                        [P, KB, 16, 2])

                def gsl(k):
                    ap = gt_t[:, :, 32 * k:32 * k + 32]
                    if blend1x:
                        return ap
                    return ap.rearrange("p k (a b) -> p k a b", b=2)

                def o4(ap):
                    if blend1x:
                        return ap
                    return ap.rearrange("p k (a b) -> p k a b", b=2)

                V.tensor_tensor(out=o4(ot[:]), in0=gsl(0), in1=msl(0),
                                op=AF.mult)
                for k in range(1, 4):
                    V.tensor_tensor(out=o4(tm[:]), in0=gsl(k), in1=msl(k),
                                    op=AF.mult)
                    V.tensor_tensor(out=ot[:], in0=ot[:], in1=tm[:],
                                    op=AF.add)

                if j % 2 == 1:
                    nc.sync.dma_start(
                        out=out_d[b, j - 1:j + 1].rearrange(
                            "a p k c -> p a (k c)"),
                        in_=ot2[:].rearrange("p a k c -> p a (k c)"))

    nc.compile()
    return nc


# In-call end offsets that may terminate a sub-gather under any of the sub
# patterns in use ([1024]*8 and [1792]*4+[1024]). The permutation pins a
# row-255 pixel at each, so the firmware's trailing-negative-index truncation
# can never fire regardless of pattern.
_EXTRA_ENDS = (1791, 3583, 5375)


def _perm4():
    """Stream permutation: slot m holds pixel perm[m]. Pins a row-255 pixel
    (y0 = 255 -> idx >= 0) at the last slot of every sub-gather (involution)."""
    ends = set()
    for s in range(HW // 1024):
        ends.add(1024 * s + 1023)
    for j in range(NCALL):
        for e in _EXTRA_ENDS:
            ends.add(8192 * j + e)
    pool = set(255 * W + k for k in range(W))
    ends_need = sorted(ends - pool)
    donors = sorted(pool - ends)
    assert len(donors) >= len(ends_need)
    perm = np.arange(HW)
    for e, d in zip(ends_need, donors):
        perm[e], perm[d] = d, e
    return perm


def _host_tables(perm=None):
    import jax
    import jax.numpy as jnp

    with jax.default_device(jax.devices('cpu')[0]):
        xs = np.asarray(jnp.linspace(-1.0, 1.0, W), dtype=np.float32)
        ys = np.asarray(jnp.linspace(-1.0, 1.0, H), dtype=np.float32)

    p = np.arange(P)[:, None]
    col = np.arange(Q)[None, :]
    n_o = 128 * col + p                       # out-layout pixel id
    f = np.arange(FW)[None, :]
    i_w = 16 * f + (p % 16) + (HW // 2) * (p // 64)   # wrapped pixel id
    if perm is not None:
        n_o = perm[n_o]
        i_w = perm[i_w]
    xg_o = xs[n_o % W].astype(np.float32)
    yg_o = ys[n_o // W].astype(np.float32)
    xg_w = xs[i_w % W].astype(np.float32)
    yg_w = ys[i_w // W].astype(np.float32)
    return xg_o, yg_o, xg_w, yg_w


def _build_qimg(img_core):
    """img_core [S,256,256,32] f32 -> fp16 corner-block table.

    OV: 256B blocks q[y, t, r, c in {0,1}, :] = img[y+r, 2t+c] (clamped),
    plus one zero pad block; gather elements span blocks u, u+1.
    Else: 512B elements q[y, t, r, c in {0..3}, :].
    """
    ns = img_core.shape[0]
    if OV:
        imgp = np.pad(img_core, ((0, 0), (0, 1), (0, 1), (0, 0)),
                      mode="edge").astype(np.float16)
        q = np.empty((ns, H, W // 2, 2, 2, C), np.float16)
        for r in range(2):
            for c in range(2):
                q[:, :, :, r, c, :] = imgp[:, r:r + H, c:c + W:2, :]
        q = q.reshape(ns, NU, ELEM // 2)
        pad = np.zeros((ns, 1, ELEM // 2), np.float16)
        return np.ascontiguousarray(np.concatenate([q, pad], axis=1))
    imgp = np.pad(img_core, ((0, 0), (0, 1), (0, 2), (0, 0)), mode="edge")
    imgp = imgp.astype(np.float16)
    q = np.empty((ns, H, W // 2, 2, 4, C), np.float16)
    for r in range(2):
        for c in range(4):
            q[:, :, :, r, c, :] = imgp[:, r:r + H, c:c + 2 * (W // 2):2, :]
    return np.ascontiguousarray(q.reshape(ns, NU, ELEM))


def _in_maps_v3(image, theta):
    xg_o, yg_o, xg_w, yg_w = _host_tables()
    in_maps = []
    for c in range(N_CORES):
        th_core = theta[c * S:(c + 1) * S]
        q = _build_qimg(image[c * S:(c + 1) * S])
        m = {
            "xg_o": xg_o, "yg_o": yg_o, "xg_w": xg_w, "yg_w": yg_w,
            "th_o": np.ascontiguousarray(
                np.tile(th_core.reshape(1, 6 * S), (P, 1)), dtype=np.float32),
            "th_w": np.ascontiguousarray(
                th_core[(np.arange(P) % 64) // 16], dtype=np.float32),
            "tok": np.zeros((P, 32), np.float32),
        }
        for b in range(S):
            m[f"qimg{b}"] = q[b]
        in_maps.append(m)
    return in_maps


def _build_qimg_v4(img_core):
    """img_core [S,256,256,32] f32 -> fp16 table q[y*256+x] =
    imgp[y:y+2, x:x+2, :] (256B elements, edge-padded)."""
    ns = img_core.shape[0]
    imgp = np.pad(img_core, ((0, 0), (0, 1), (0, 1), (0, 0)),
                  mode="edge").astype(np.float16)
    q = np.empty((ns, H, W, 2, 2, C), np.float16)
    for r in range(2):
        for cc in range(2):
            q[:, :, :, r, cc, :] = imgp[:, r:r + H, cc:cc + W, :]
    return np.ascontiguousarray(q.reshape(ns, NU4, ELEM4))


def _check_perm_safety(theta, perm):
    """Every sub-gather's final stream slot must produce idx >= 0, i.e.
    y0 >= 128 for that pixel (firmware truncates trailing negatives)."""
    import jax
    import jax.numpy as jnp
    with jax.default_device(jax.devices('cpu')[0]):
        xs = np.asarray(jnp.linspace(-1.0, 1.0, W), dtype=np.float64)
        ys = np.asarray(jnp.linspace(-1.0, 1.0, H), dtype=np.float64)
    ends = list(1024 * np.arange(HW // 1024) + 1023)
    for j in range(NCALL):
        ends.extend(8192 * j + e for e in _EXTRA_ENDS)
    end_pix = perm[np.asarray(sorted(set(ends)))]
    xg = xs[end_pix % W]
    yg = ys[end_pix // W]
    th = theta.astype(np.float64)
    y = 0.5 * (th[:, 3:4] * xg[None, :] + th[:, 4:5] * yg[None, :]
               + th[:, 5:6] + 1.0) * H
    assert (y >= 129.0).all(), (
        f"v4 sub-gather end pixel too low: min y={y.min():.2f}")


def _in_maps_v4(image, theta):
    perm = _perm4()
    _check_perm_safety(theta, perm)
    xg_o, yg_o, xg_w, yg_w = _host_tables(perm)
    in_maps = []
    for c in range(N_CORES):
        th_core = theta[c * S:(c + 1) * S]
        q = _build_qimg_v4(image[c * S:(c + 1) * S])
        m = {
            "xg_o": xg_o, "yg_o": yg_o, "xg_w": xg_w, "yg_w": yg_w,
            "th_o": np.ascontiguousarray(
                np.tile(th_core.reshape(1, 6 * S), (P, 1)), dtype=np.float32),
            "th_w": np.ascontiguousarray(
                th_core[(np.arange(P) % 64) // 16], dtype=np.float32),
            "tok": np.zeros((P, 32), np.float32),
        }
        for b in range(S):
            m[f"qimg{b}"] = q[b]
        in_maps.append(m)
    return in_maps


def _in_maps(image, theta):
    if VERSION == 4:
        return _in_maps_v4(image, theta)
    return _in_maps_v3(image, theta)


def kernel(image: np.ndarray, theta: np.ndarray) -> np.ndarray:
    image = np.ascontiguousarray(image, dtype=np.float32)
    theta = np.ascontiguousarray(theta, dtype=np.float32)
    assert image.shape == (B_TOTAL, H, W, C) and theta.shape == (B_TOTAL, 6)

    if "nc" not in _COMPILED:
        _COMPILED["nc"] = _build_nc_v4() if VERSION == 4 else _build_nc()
    nc = _COMPILED["nc"]

    in_maps = _in_maps(image, theta)
    res = run_bass_kernel_spmd(nc, in_maps, core_ids=list(range(N_CORES)))

    perm = _perm4() if VERSION == 4 else None
    out = np.empty((B_TOTAL, H, W, C), np.float32)
    for c in range(N_CORES):
        raw = res.results[c]["out"]            # [S, NCALL, P, KB, C] fp16
        slots = raw.transpose(0, 1, 3, 2, 4).reshape(S, HW, C)
        if perm is not None:
            slots = slots[:, perm, :]          # pixel n lives at slot perm[n]
        out[c * S:(c + 1) * S] = slots.reshape(S, H, W, C).astype(np.float32)
    return out

